# revision 6
# baseline (speedup 1.0000x reference)
"""CountHistogram Trainium2 kernel (v5: host-side masking + 4-engine split).

Reference computation:
    bins = trunc((simmat + 1.00001) / 2 * 29)            (values in [0, 30))
    w[b,q,d] = (dtoks[b,d] != -1) & (qtoks[b,q] != -1)
    hist[b,c,q,n] = sum_d w[b,q,d] * (bins[b,c,q,d] == n)

Key idea vs v4: the device computes the UNMASKED histogram; masking is
reconstructed on the host (subtract the ~1% of d-columns with dtok==-1,
zero rows with qtok==-1). This removes the penalty-surface matmul and the
PSUM-operand convert, freeing DVE/PE cycles.

Per batch b the tile is [128 rows = (c,q), 2048 = d] fp32.
1) DVE tensor_scalar (2x): u16 = int16_rne(sim*14.5 + (1.00001*14.5 - 0.5))
   == floor(sim*14.5 + 1.00001*14.5)  (modulo rare half-way ties).
2) 30 bins split across engines:
   - PE_BINS:  DMA-transpose u16 -> ut [128 d, 16*128 rows] int16; per bin a
     4x-mode DVE is_equal mask (bf16), reduced over the 128 d partitions by
     PE matmuls (one-hot lhsT routes bin n to PSUM row n); PSUM [32, 2048]
     holds 16 chunk-sums per row, folded on host.
   - GP_BINS:  GPSIMD tensor_scalar(is_equal, accum) passes.
   - ACT_BINS: ACT activation(Relu, bias=-m, accum) computes
     S_m = sum_d relu(u16 - m); hist = second difference of S (host).
All counts are exact small integers in fp32.
"""

import sys

import numpy as np

sys.path.insert(0, "/opt/trn_rl_repo")

NBINS = 30
B, C, Q, D = 128, 4, 32, 2048
NCORES = 8
BS = B // NCORES  # batches per core
ROWS = C * Q  # 128

# engine split: PE takes low bins, GPSIMD middle, ACT top (S-curves)
N_ACT = 8
N_GP = 0
N_PE = NBINS - N_ACT - N_GP  # 22
PE_BINS = list(range(N_PE))
GP_BINS = list(range(N_PE, N_PE + N_GP))
ACT_LO = N_PE + N_GP  # first ACT bin
ACT_SLEV = list(range(ACT_LO - 1, NBINS))  # S levels m = ACT_LO-1 .. 29

# fp32 constants of the device bin transform: u = rint(x*14.5 + C_ADD)
C0 = np.float32(np.float32(1.00001) * np.float32(14.5))
C_ADD = np.float32(C0 - np.float32(0.5))

_CACHE = {}
LAST_RESULTS = None


def _build():
    import concourse.bacc as bacc  # noqa
    import concourse.bass as bass
    import concourse.mybir as mybir
    import concourse.tile as tile

    A = mybir.AluOpType
    dt = mybir.dt
    AF = mybir.ActivationFunctionType

    nc = bacc.Bacc("TRN2", target_bir_lowering=False, debug=False, num_devices=NCORES)

    sim = nc.dram_tensor("simmat", [BS, ROWS, D], dt.float32, kind="ExternalInput")
    h_pe = nc.dram_tensor("h_pe", [BS, 32, ROWS], dt.float32, kind="ExternalOutput")
    h_act = nc.dram_tensor(
        "h_act", [BS, ROWS, len(ACT_SLEV)], dt.float32, kind="ExternalOutput"
    )
    h_gp = nc.dram_tensor(
        "h_gp", [BS, ROWS, max(N_GP, 1)], dt.float32, kind="ExternalOutput"
    )

    with tile.TileContext(nc) as tc:
        with (
            tc.tile_pool(name="const", bufs=1) as cpool,
            tc.tile_pool(name="sbuf", bufs=3) as pool,
            tc.tile_pool(name="mask", bufs=3) as mpool,
            tc.tile_pool(name="small", bufs=3) as spool,
            tc.tile_pool(name="psumh", bufs=2, space="PSUM") as psumh,
        ):
            # E[:, j] = 1 iff j == 31; lhsT for bin n = E[:, 31-n : 63-n]
            emat = cpool.tile([128, 63], dt.bfloat16)
            nc.vector.memset(emat[:], 0.0)
            nc.vector.memset(emat[:, 31:32], 1.0)
            biases = cpool.tile([128, len(ACT_SLEV)], dt.float32)
            for i, m in enumerate(ACT_SLEV):
                nc.vector.memset(biases[:, i : i + 1], float(-m))

            for b in range(BS):
                st = pool.tile([ROWS, D], dt.float32, tag="sim")
                nc.sync.dma_start(st[:], sim[b, :, :])

                # u16 = rint(sim*14.5 + C_ADD)  (DVE 2x, fp32 src)
                u16 = pool.tile([ROWS, D], dt.int16, tag="u16")
                nc.vector.tensor_scalar(
                    u16[:], st[:], 14.5, float(C_ADD), A.mult, A.add
                )

                # --- PE route ---
                ut = pool.tile([128, 16, 128], dt.int16, tag="ut")
                nc.sync.dma_start_transpose(ut[:], u16[:])
                ph = psumh.tile([32, D], dt.float32, tag="ph")
                utf = ut[:].rearrange("p a b -> p (a b)")
                for i, n in enumerate(PE_BINS):
                    mt = mpool.tile([128, D], dt.bfloat16, tag="mt")
                    nc.vector.tensor_scalar(mt[:], utf, float(n), None, A.is_equal)
                    for c in range(4):
                        nc.tensor.matmul(
                            ph[:, c * 512 : (c + 1) * 512],
                            emat[:, 31 - n : 63 - n],
                            mt[:, c * 512 : (c + 1) * 512],
                            start=(i == 0),
                            stop=(i == len(PE_BINS) - 1),
                            skip_group_check=True,
                        )
                # fold the 16 chunks: ph free is (chunk, row) -> [32, ROWS]
                ph_view = ph[:].rearrange("p (a b) -> p b a", a=16, b=ROWS)
                hrow = spool.tile([32, ROWS], dt.float32, tag="hrow")
                nc.vector.tensor_reduce(
                    hrow[:], ph_view, mybir.AxisListType.X, A.add
                )
                nc.sync.dma_start(h_pe[b, :, :], hrow[:])

                # --- GPSIMD route ---
                if N_GP:
                    gm = mpool.tile([ROWS, D], dt.int16, tag="gm")
                    ga = spool.tile([ROWS, N_GP], dt.float32, tag="ga")
                    for i, n in enumerate(GP_BINS):
                        nc.gpsimd.tensor_scalar(
                            gm[:],
                            u16[:],
                            float(n),
                            None,
                            A.is_equal,
                            A.add,
                            accum_out=ga[:, i : i + 1],
                        )
                    nc.sync.dma_start(h_gp[b, :, :], ga[:])

                # --- ACT route: S_m = sum_d relu(u16 - m) ---
                ma = mpool.tile([ROWS, D], dt.int16, tag="ma")
                sa = spool.tile([ROWS, len(ACT_SLEV)], dt.float32, tag="sa")
                for i, m in enumerate(ACT_SLEV):
                    nc.scalar.activation(
                        ma[:],
                        u16[:],
                        AF.Relu,
                        bias=biases[:, i : i + 1],
                        scale=1.0,
                        accum_out=sa[:, i : i + 1],
                    )
                nc.sync.dma_start(h_act[b, :, :], sa[:])

    nc.compile()
    return nc


def _get_nc():
    if "nc" not in _CACHE:
        _CACHE["nc"] = _build()
    return _CACHE["nc"]


def _device_bins(x):
    """Replicate the device transform: rint(fp32(x)*14.5 + C_ADD)."""
    y = x.astype(np.float32) * np.float32(14.5) + C_ADD
    return np.rint(y).astype(np.int32)


def kernel(simmat, dlens, dtoks, qtoks):
    global LAST_RESULTS
    from concourse.bass_utils import run_bass_kernel_spmd

    simmat = np.ascontiguousarray(simmat, dtype=np.float32)
    dtoks = np.asarray(dtoks)
    qtoks = np.asarray(qtoks)

    nc = _get_nc()

    sim_rows = simmat.reshape(B, ROWS, D)
    in_maps = []
    for core in range(NCORES):
        lo, hi = core * BS, (core + 1) * BS
        in_maps.append({"simmat": np.ascontiguousarray(sim_rows[lo:hi])})

    res = run_bass_kernel_spmd(nc, in_maps, core_ids=list(range(NCORES)))
    LAST_RESULTS = res

    full = np.zeros((B, ROWS, NBINS), np.float32)
    for core in range(NCORES):
        lo = core * BS
        o = res.results[core]
        # PE bins: h_pe [BS, 32, ROWS] -> [BS, ROWS, bins]
        pe = o["h_pe"]
        full[lo : lo + BS, :, :N_PE] = pe[:, :N_PE, :].transpose(0, 2, 1)
        # GPSIMD bins
        if N_GP:
            full[lo : lo + BS, :, N_PE : N_PE + N_GP] = o["h_gp"][:, :, :N_GP]
        # ACT bins: hist_n = S_{n-1} - 2 S_n + S_{n+1}, S_30 = 0
        S = o["h_act"]  # [BS, ROWS, N_ACT+1] for m = ACT_LO-1..29
        Sx = np.concatenate([S, np.zeros((BS, ROWS, 1), np.float32)], axis=2)
        full[lo : lo + BS, :, ACT_LO:] = (
            Sx[:, :, 0:N_ACT] - 2 * Sx[:, :, 1 : N_ACT + 1] + Sx[:, :, 2:]
        )

    # --- host-side masking corrections ---
    # 1) subtract contributions of d-columns with dtok == -1
    for b in range(B):
        md = np.nonzero(dtoks[b] == -1)[0]
        if md.size == 0:
            continue
        bins_m = _device_bins(simmat[b, :, :, md])  # [C, Q, nm]
        bins_m = np.clip(bins_m, 0, NBINS - 1)
        rows_idx = np.repeat(np.arange(ROWS), md.size)
        np.subtract.at(
            full[b], (rows_idx, bins_m.reshape(ROWS, md.size).reshape(-1)), 1.0
        )
    # 2) zero rows with qtok == -1 (row index r = c*Q + q)
    qbad = qtoks == -1  # [B, Q]
    if qbad.any():
        mask_rows = np.tile(qbad, (1, C))  # [B, C*Q] with r = c*Q + q
        full[mask_rows] = 0.0

    return full.reshape(B, C, Q, NBINS).astype(np.float32)


# revision 7
# speedup vs baseline: 1.0011x; 1.0011x over previous
"""CountHistogram Trainium2 kernel (v5: host-side masking + 4-engine split).

Reference computation:
    bins = trunc((simmat + 1.00001) / 2 * 29)            (values in [0, 30))
    w[b,q,d] = (dtoks[b,d] != -1) & (qtoks[b,q] != -1)
    hist[b,c,q,n] = sum_d w[b,q,d] * (bins[b,c,q,d] == n)

Key idea vs v4: the device computes the UNMASKED histogram; masking is
reconstructed on the host (subtract the ~1% of d-columns with dtok==-1,
zero rows with qtok==-1). This removes the penalty-surface matmul and the
PSUM-operand convert, freeing DVE/PE cycles.

Per batch b the tile is [128 rows = (c,q), 2048 = d] fp32.
1) DVE tensor_scalar (2x): u16 = int16_rne(sim*14.5 + (1.00001*14.5 - 0.5))
   == floor(sim*14.5 + 1.00001*14.5)  (modulo rare half-way ties).
2) 30 bins split across engines:
   - PE_BINS:  DMA-transpose u16 -> ut [128 d, 16*128 rows] int16; per bin a
     4x-mode DVE is_equal mask (bf16), reduced over the 128 d partitions by
     PE matmuls (one-hot lhsT routes bin n to PSUM row n); PSUM [32, 2048]
     holds 16 chunk-sums per row, folded on host.
   - GP_BINS:  GPSIMD tensor_scalar(is_equal, accum) passes.
   - ACT_BINS: ACT activation(Relu, bias=-m, accum) computes
     S_m = sum_d relu(u16 - m); hist = second difference of S (host).
All counts are exact small integers in fp32.
"""

import sys

import numpy as np

sys.path.insert(0, "/opt/trn_rl_repo")

NBINS = 30
B, C, Q, D = 128, 4, 32, 2048
NCORES = 8
BS = B // NCORES  # batches per core
ROWS = C * Q  # 128

# engine split: PE takes low bins, GPSIMD middle, ACT top (S-curves)
N_ACT = 8
N_GP = 0
N_PE = NBINS - N_ACT - N_GP  # 22
PE_BINS = list(range(N_PE))
GP_BINS = list(range(N_PE, N_PE + N_GP))
ACT_LO = N_PE + N_GP  # first ACT bin
ACT_SLEV = list(range(ACT_LO - 1, NBINS))  # S levels m = ACT_LO-1 .. 29

# fp32 constants of the device bin transform: u = rint(x*14.5 + C_ADD)
C0 = np.float32(np.float32(1.00001) * np.float32(14.5))
C_ADD = np.float32(C0 - np.float32(0.5))

_CACHE = {}
LAST_RESULTS = None


def _build():
    import concourse.bacc as bacc  # noqa
    import concourse.bass as bass
    import concourse.mybir as mybir
    import concourse.tile as tile

    A = mybir.AluOpType
    dt = mybir.dt
    AF = mybir.ActivationFunctionType

    nc = bacc.Bacc("TRN2", target_bir_lowering=False, debug=False, num_devices=NCORES)

    sim = nc.dram_tensor("simmat", [BS, ROWS, D], dt.float32, kind="ExternalInput")
    h_pe = nc.dram_tensor("h_pe", [BS, 32, ROWS], dt.float32, kind="ExternalOutput")
    h_act = nc.dram_tensor(
        "h_act", [BS, ROWS, len(ACT_SLEV)], dt.float32, kind="ExternalOutput"
    )
    h_gp = nc.dram_tensor(
        "h_gp", [BS, ROWS, max(N_GP, 1)], dt.float32, kind="ExternalOutput"
    )

    with tile.TileContext(nc) as tc:
        with (
            tc.tile_pool(name="const", bufs=1) as cpool,
            tc.tile_pool(name="sbuf", bufs=3) as pool,
            tc.tile_pool(name="mask", bufs=3) as mpool,
            tc.tile_pool(name="small", bufs=3) as spool,
            tc.tile_pool(name="psumh", bufs=2, space="PSUM") as psumh,
        ):
            # E[:, j] = 1 iff j == 31; lhsT for bin n = E[:, 31-n : 63-n]
            emat = cpool.tile([128, 63], dt.bfloat16)
            nc.vector.memset(emat[:], 0.0)
            nc.vector.memset(emat[:, 31:32], 1.0)
            biases = cpool.tile([128, len(ACT_SLEV)], dt.float32)
            for i, m in enumerate(ACT_SLEV):
                nc.vector.memset(biases[:, i : i + 1], float(-m))

            for b in range(BS):
                st = pool.tile([ROWS, D], dt.float32, tag="sim")
                nc.sync.dma_start(st[:], sim[b, :, :])

                # u16 = rint(sim*14.5 + C_ADD)  (DVE 2x, fp32 src)
                u16 = pool.tile([ROWS, D], dt.int16, tag="u16")
                nc.vector.tensor_scalar(
                    u16[:], st[:], 14.5, float(C_ADD), A.mult, A.add
                )

                # --- PE route ---
                ut = pool.tile([128, 16, 128], dt.int16, tag="ut")
                nc.sync.dma_start_transpose(ut[:], u16[:])
                ph = psumh.tile([32, D], dt.float32, tag="ph")
                utf = ut[:].rearrange("p a b -> p (a b)")
                for i, n in enumerate(PE_BINS):
                    mt = mpool.tile([128, D], dt.bfloat16, tag="mt")
                    nc.vector.tensor_scalar(mt[:], utf, float(n), None, A.is_equal)
                    for c in range(4):
                        nc.tensor.matmul(
                            ph[:, c * 512 : (c + 1) * 512],
                            emat[:, 31 - n : 63 - n],
                            mt[:, c * 512 : (c + 1) * 512],
                            start=(i == 0),
                            stop=(i == len(PE_BINS) - 1),
                            skip_group_check=True,
                        )
                # fold the 16 chunks: ph free is (chunk, row) -> [32, ROWS]
                ph_view = ph[:].rearrange("p (a b) -> p b a", a=16, b=ROWS)
                hrow = spool.tile([32, ROWS], dt.float32, tag="hrow")
                nc.vector.tensor_reduce(
                    hrow[:], ph_view, mybir.AxisListType.X, A.add
                )
                nc.sync.dma_start(h_pe[b, :, :], hrow[:])

                # --- GPSIMD route ---
                if N_GP:
                    gm = mpool.tile([ROWS, D], dt.int16, tag="gm")
                    ga = spool.tile([ROWS, N_GP], dt.float32, tag="ga")
                    for i, n in enumerate(GP_BINS):
                        nc.gpsimd.tensor_scalar(
                            gm[:],
                            u16[:],
                            float(n),
                            None,
                            A.is_equal,
                            A.add,
                            accum_out=ga[:, i : i + 1],
                        )
                    nc.sync.dma_start(h_gp[b, :, :], ga[:])

                # --- ACT route: S_m = sum_d relu(u16 - m) ---
                ma = mpool.tile([ROWS, D], dt.int16, tag="ma")
                sa = spool.tile([ROWS, len(ACT_SLEV)], dt.float32, tag="sa")
                for i, m in enumerate(ACT_SLEV):
                    nc.scalar.activation(
                        ma[:],
                        u16[:],
                        AF.Relu,
                        bias=biases[:, i : i + 1],
                        scale=1.0,
                        accum_out=sa[:, i : i + 1],
                    )
                nc.sync.dma_start(h_act[b, :, :], sa[:])

    nc.compile()
    return nc


def _get_nc():
    if "nc" not in _CACHE:
        _CACHE["nc"] = _build()
    return _CACHE["nc"]


def _device_bins(x):
    """Replicate the device transform: rint(fp32(x)*14.5 + C_ADD)."""
    y = x.astype(np.float32) * np.float32(14.5) + C_ADD
    return np.rint(y).astype(np.int32)


def kernel(simmat, dlens, dtoks, qtoks):
    global LAST_RESULTS
    from concourse.bass_utils import run_bass_kernel_spmd

    simmat = np.ascontiguousarray(simmat, dtype=np.float32)
    dtoks = np.asarray(dtoks)
    qtoks = np.asarray(qtoks)

    nc = _get_nc()

    sim_rows = simmat.reshape(B, ROWS, D)
    in_maps = []
    for core in range(NCORES):
        lo, hi = core * BS, (core + 1) * BS
        in_maps.append({"simmat": np.ascontiguousarray(sim_rows[lo:hi])})

    res = run_bass_kernel_spmd(nc, in_maps, core_ids=list(range(NCORES)))
    LAST_RESULTS = res

    full = np.zeros((B, ROWS, NBINS), np.float32)
    for core in range(NCORES):
        lo = core * BS
        o = res.results[core]
        # PE bins: h_pe [BS, 32, ROWS] -> [BS, ROWS, bins]
        pe = o["h_pe"]
        full[lo : lo + BS, :, :N_PE] = pe[:, :N_PE, :].transpose(0, 2, 1)
        # GPSIMD bins
        if N_GP:
            full[lo : lo + BS, :, N_PE : N_PE + N_GP] = o["h_gp"][:, :, :N_GP]
        # ACT bins: hist_n = S_{n-1} - 2 S_n + S_{n+1}, S_30 = 0
        S = o["h_act"]  # [BS, ROWS, N_ACT+1] for m = ACT_LO-1..29
        Sx = np.concatenate([S, np.zeros((BS, ROWS, 1), np.float32)], axis=2)
        full[lo : lo + BS, :, ACT_LO:] = (
            Sx[:, :, 0:N_ACT] - 2 * Sx[:, :, 1 : N_ACT + 1] + Sx[:, :, 2:]
        )

    # --- host-side masking corrections ---
    # 1) subtract contributions of d-columns with dtok == -1
    for b in range(B):
        md = np.nonzero(dtoks[b] == -1)[0]
        if md.size == 0:
            continue
        bins_m = _device_bins(simmat[b][:, :, md])  # [C, Q, nm]
        bins_m = np.clip(bins_m, 0, NBINS - 1)
        rows_idx = np.repeat(np.arange(ROWS), md.size)
        np.subtract.at(
            full[b], (rows_idx, bins_m.reshape(ROWS, md.size).reshape(-1)), 1.0
        )
    # 2) zero rows with qtok == -1 (row index r = c*Q + q)
    qbad = qtoks == -1  # [B, Q]
    if qbad.any():
        mask_rows = np.tile(qbad, (1, C))  # [B, C*Q] with r = c*Q + q
        full[mask_rows] = 0.0

    return full.reshape(B, C, Q, NBINS).astype(np.float32)
